# revision 30
# baseline (speedup 1.0000x reference)
"""Trainium2 Bass kernel for CrossModalAttentionImproved.

Single-head cross attention + FFN transformer block:
  q = Xq@Wq+bq; k = Xk@Wk+bk; v = Xk@Wv+bv
  attn = softmax(q k^T / sqrt(D)); ctx = attn@v
  out = LN(Xq + ctx@Wo + bo; g1,b1)
  h = gelu(LN(out@W1 + bf1; gf,bf))
  y = LN(out + h@W2 + bf2; g2,b2)

Sharding: data-parallel over batch. B=16 across 8 cores x 2 sequential
elements per core (one batch element per program iteration). Params
replicated.

Algebraic folds (host precomputes, exact):
  - scores = q k^T = Xq (Wq Wk^T) Xk^T: one fused projection M1 = Xq@Wqk,
    and Xk^T (the raw fp8 input) is the scores operand directly -> the
    K projection disappears. bk cancels in softmax; bq becomes a per-key
    bias column folded into the exp (zero here).
  - ctx@Wo = attn @ (V Wo) with VWo = Xk@(Wv Wo): the Wo stage, the ctx
    PE-transposes and fp8 requant all disappear.
  - fp8 weights are scaled x64 host-side (escapes e4m3 subnormals);
    the scale is divided out via the folded exp constant / pre-scaled
    residual (Xq*64 from host), so no extra device ops.

Layout strategy (attn matmuls fp8 DoubleRow, FFN bf16, fp32 PSUM):
  - host pre-transposes Xq/Xk to [D,N] fp8
  - M1T produced transposed [D,N] via PE; V=Xk@WvWo natural [N,D] with a
    ones column -> softmax sums land as a per-partition column
  - scoresT[k,q] per k-tile; exp on ACT (scale folded); eT fp8
  - ctx eviction: reciprocal + fused (ps*recip + Xq') on DVE, LN1 via
    bn_stats + DVE-bit-trick rsqrt
  - FFN1 natural out; LN+GELU fused into a single ACT op
  - h bounced through DRAM with DMA-transpose (2-byte xbar) for FFN2
"""

import sys

if '/opt/trn_rl_repo' not in sys.path:
    sys.path.insert(0, '/opt/trn_rl_repo')

import math
from contextlib import ExitStack

import numpy as np
import ml_dtypes

import concourse.bass as bass
import concourse.tile as tile
from concourse import bacc, mybir
from concourse import bass2jax

F32 = mybir.dt.float32
BF16 = mybir.dt.bfloat16
F8 = mybir.dt.float8e4
DR = mybir.MatmulPerfMode.DoubleRow
U32 = mybir.dt.uint32
Alu = mybir.AluOpType
Act = mybir.ActivationFunctionType

EPS = 1e-5
P = 128
SW = 64.0          # host-side scale on fused fp8 weights (and Xq residual)


# ---------------------------------------------------------------------------
# device program
# ---------------------------------------------------------------------------

def build_program(N=2048, D=768, H=3072, QB=512, n_elems=1, n_reps=1,
                  nontrivial=frozenset(), skip=frozenset()):
    """Build + compile the per-core program (n_elems batch elements).

    nontrivial: subset of {bq, bvo, bf1, bf2, g1b1, gfbf, g2b2} naming
    affine params that are not identity and need real ops emitted.
    skip: phase-ablation set for benchmarking, subset of
    {proj, scores, ctx, ffn1, ffn2}.
    """
    DC = D // P          # d chunks (6)
    HC = H // P          # h chunks (24)
    RT = N // P          # row tiles (16)
    NB = N // QB         # q blocks (4)
    SB = QB // P         # subtiles per block (4)
    F1N = min(512, H)    # FFN1 n-chunk width
    F1C = H // F1N       # FFN1 n-chunks (6)
    KC = DC // 2         # fp8 DR d chunk pairs (3)
    scale = 1.0 / (math.sqrt(D) * SW)
    has_ck = "bq" in nontrivial      # per-key score bias column
    VW = D + 16          # v tile row width (D + ones col + optional ck)

    def slices(total):
        out, lo = [], 0
        while lo < total:
            hi = min(lo + 512, total)
            out.append((lo, hi))
            lo = hi
        return out

    D_SL = slices(D)          # [(0,512),(512,768)]
    D1_SL = slices(D + 1)     # [(0,512),(512,769)]

    nc = bacc.Bacc("TRN2", target_bir_lowering=False, debug=False,
                   num_devices=8)

    # ---- DRAM I/O -----------------------------------------------------
    l_xqT = [nc.dram_tensor(f"xqT{e}", [D, N], F8, kind="ExternalInput")
             for e in range(n_elems)]
    l_xkT = [nc.dram_tensor(f"xkT{e}", [D, N], F8, kind="ExternalInput")
             for e in range(n_elems)]
    l_xq = [nc.dram_tensor(f"xq{e}", [N, D], F32, kind="ExternalInput")
            for e in range(n_elems)]
    d_wqk = nc.dram_tensor("wqk", [D, D], F8, kind="ExternalInput")
    d_wvo = nc.dram_tensor("wvo", [D, D], F8, kind="ExternalInput")
    d_w1 = nc.dram_tensor("w1", [D, H], BF16, kind="ExternalInput")
    d_w2 = nc.dram_tensor("w2", [H, D], BF16, kind="ExternalInput")
    dram_aff = {}
    l_ck = []
    if has_ck:
        l_ck = [nc.dram_tensor(f"ck{e}", [N], F32, kind="ExternalInput")
                for e in range(n_elems)]
    for nm, sz in (("bvo", D), ("bf1", H), ("bf2", D)):
        if nm in nontrivial:
            dram_aff[nm] = nc.dram_tensor(nm, [sz], BF16, kind="ExternalInput")
    for nm, sz in (("g1b1", D), ("gfbf", H), ("g2b2", D)):
        if nm in nontrivial:
            dram_aff[nm + "_g"] = nc.dram_tensor(nm + "_g", [sz], F32,
                                                 kind="ExternalInput")
            dram_aff[nm + "_b"] = nc.dram_tensor(nm + "_b", [sz], F32,
                                                 kind="ExternalInput")
    l_y = [nc.dram_tensor(f"y{e}", [N, D], F32, kind="ExternalOutput")
           for e in range(n_elems)]
    # internal scratch
    l_outf = [nc.dram_tensor(f"out_f32_{e}", [N, D], F32)
              for e in range(n_elems)]
    l_outb = [nc.dram_tensor(f"out_b16_{e}", [N, D], BF16)
              for e in range(n_elems)]
    l_h = [nc.dram_tensor(f"h_b16_{e}", [N, H], BF16)
           for e in range(n_elems)]

    # bn_stats subgroup sizes
    bn_d = math.gcd(512, D)      # 256 for 768
    bn_dn = D // bn_d

    def emit_rsqrt(pool, nc, var_ap, tag):
        """rstd[P,1] f32 = 1/sqrt(var+EPS), DVE only (no ACT tables)."""
        ve = pool.tile([P, 1], F32, tag=f"rs_ve_{tag}", bufs=2)
        nc.vector.tensor_scalar_add(ve, var_ap, EPS)
        y = pool.tile([P, 1], F32, tag=f"rs_y_{tag}", bufs=2)
        nc.vector.tensor_scalar(
            out=y.bitcast(U32), in0=ve.bitcast(U32),
            scalar1=1, scalar2=0xFFFFFFFF,
            op0=Alu.logical_shift_right, op1=Alu.bitwise_xor)
        nc.vector.tensor_scalar(
            out=y.bitcast(U32), in0=y.bitcast(U32),
            scalar1=0xA0C8A620, scalar2=None, op0=Alu.subtract)
        t = pool.tile([P, 1], F32, tag=f"rs_t_{tag}", bufs=2)
        for _ in range(1):
            nc.vector.tensor_mul(t, y, y)
            nc.vector.tensor_mul(t, t, ve)
            nc.vector.tensor_scalar(out=t, in0=t, scalar1=-0.5, scalar2=1.5,
                                    op0=Alu.mult, op1=Alu.add)
            nc.vector.tensor_mul(y, y, t)
        return y

    def load_rep(pool, nc, dram, sz, tag):
        """Broadcast a [sz] dram vector across partitions -> [P, sz] tile."""
        t = pool.tile([P, sz], dram.dtype, tag=tag, bufs=1, name=tag)
        ap = dram.ap()
        bcast = bass.AP(tensor=ap.tensor, offset=ap.offset,
                        ap=[[0, P]] + list(ap.ap))
        nc.gpsimd.dma_start(out=t, in_=bcast)
        return t

    with tile.TileContext(nc) as tc, ExitStack() as octx:
        pers = octx.enter_context(tc.tile_pool(name="pers", bufs=1))

        # fused attention weights, DR pair layout: d = c*256 + i*128 + p
        wqk = pers.tile([P, KC, 2, D], F8)
        nc.sync.dma_start(out=wqk, in_=d_wqk.ap().rearrange(
            "(c i p) f -> p c i f", p=P, i=2))
        wvo = pers.tile([P, KC, 2, D], F8)
        nc.sync.dma_start(out=wvo, in_=d_wvo.ap().rearrange(
            "(c i p) f -> p c i f", p=P, i=2))
        # FFN weights, persistent across elements. Loaded on the Pool
        # (SWDGE) queue in per-chunk pieces so they never block the SP
        # queue's latency-critical phase-A streams; they have all of
        # phases A+B to land before first use in phase C.
        w1 = pers.tile([P, DC, H], BF16)
        for j in range(DC):
            nc.gpsimd.dma_start(out=w1[:, j, :], in_=d_w1.ap().rearrange(
                "(c p) f -> p c f", p=P)[:, j, :])
        w2 = pers.tile([P, HC, D], BF16)
        for j in range(0, HC, 4):
            nc.gpsimd.dma_start(
                out=w2[:, j:j + 4, :], in_=d_w2.ap().rearrange(
                    "(c p) f -> p c f", p=P)[:, j:j + 4, :])


        ones_row = None
        if any(k in nontrivial for k in ("bvo", "bf1", "bf2")):
            ones_row = pers.tile([1, P], BF16)
            nc.vector.memset(ones_row, 1.0)
        bias_rows = {}
        for nm in ("bvo", "bf1", "bf2"):
            if nm in nontrivial:
                sz = H if nm == "bf1" else D
                t = pers.tile([1, sz], BF16, tag=f"brow_{nm}",
                              name=f"brow_{nm}")
                nc.sync.dma_start(out=t, in_=dram_aff[nm].ap().rearrange(
                    "(o f) -> o f", o=1))
                bias_rows[nm] = t
        gain_reps = {}
        for nm in ("g1b1", "gfbf", "g2b2"):
            if nm in nontrivial:
                sz = H if nm == "gfbf" else D
                gain_reps[nm + "_g"] = load_rep(pers, nc, dram_aff[nm + "_g"],
                                                sz, f"grep_{nm}")
                gain_reps[nm + "_b"] = load_rep(pers, nc, dram_aff[nm + "_b"],
                                                sz, f"brep_{nm}")

        def add_bias_row(psum_ap, nm, lo, hi):
            """Accumulate broadcast bias row into psum via K=1 matmul."""
            nc.tensor.matmul(psum_ap[:, lo:hi], lhsT=ones_row,
                             rhs=bias_rows[nm][:, lo:hi],
                             start=False, stop=True)

        def post_ln_affine(nc, buf, nm):
            if nm in nontrivial:
                nc.vector.tensor_mul(buf, buf, gain_reps[nm + "_g"])
                nc.vector.tensor_add(buf, buf, gain_reps[nm + "_b"])

        iters = [(r, e) for r in range(n_reps) for e in range(n_elems)]
        for _idx, (_rep, _e) in enumerate(iters):
            _pe = f"{_rep}_{_e}"
            d_xqT, d_xkT, d_xq = l_xqT[_e], l_xkT[_e], l_xq[_e]
            d_y, d_outf, d_outb, d_h = l_y[_e], l_outf[_e], l_outb[_e], l_h[_e]
            # ==== cross-phase pool: outT (B->C) ====
            ectx = ExitStack()
            crossBC = ectx.enter_context(tc.tile_pool(name=f"crossBC{_pe}",
                                                      bufs=1))
            outT = crossBC.tile([P, DC, N], BF16)

            # ============ Phases A+B share kv8 / m1T / v tiles ============
            ab_ctx = ectx.enter_context(ExitStack())
            qkv = ab_ctx.enter_context(tc.tile_pool(name=f"qkv{_pe}", bufs=1))
            kv8 = qkv.tile([P, KC, 2, N], F8)      # Xk^T resident
            m1T = qkv.tile([P, KC, 2, N], F8)      # (Xq@Wqk)^T
            v = qkv.tile([P, RT // 2, 2, VW], F8)  # Xk@WvWo | ones
            ck_t = qkv.tile([P, RT], F32) if has_ck else None

            def qk_slot(t, m, sl):
                return t[:, m // 2, m % 2, sl]

            with ExitStack() as ctx:
                pa = ctx.enter_context(tc.tile_pool(name=f"pa{_pe}", bufs=1))
                psA = ctx.enter_context(tc.tile_pool(name=f"psA{_pe}", bufs=3,
                                                     space="PSUM"))
                psV = ctx.enter_context(tc.tile_pool(name=f"psV{_pe}", bufs=2,
                                                     space="PSUM"))

                def x_dram(d):
                    return d.ap().rearrange("(c i p) n -> p c i n", p=P, i=2)

                # kv8 loaded in column chunks so V(rt) starts early
                for cb in range(NB):
                    nc.sync.dma_start(
                        out=kv8[:, :, :, cb * QB:(cb + 1) * QB],
                        in_=x_dram(d_xkT)[:, :, :, cb * QB:(cb + 1) * QB])
                if has_ck:
                    nc.sync.dma_start(out=ck_t, in_=l_ck[_e].ap().rearrange(
                        "(t p) -> p t", p=P))

                # Interleaved per row-block: M1T chunk (phase B needs it
                # first) then this block's V row tiles (overlap xqT stream)
                KCp = 1 if "proj" in skip else KC
                for rb in range(NB):
                    xqTc = pa.tile([P, KC, 2, QB], F8, tag="xqTc",
                                   bufs=2, name="xqTc")
                    nc.sync.dma_start(out=xqTc, in_=x_dram(d_xqT)[
                        :, :, :, rb * QB:(rb + 1) * QB])
                    for m in range(DC):
                        ps = psA.tile([P, QB], F32, tag="psA")
                        for k in range(KCp):
                            nc.tensor.matmul(
                                ps,
                                lhsT=wqk[:, k, :, m * P:(m + 1) * P],
                                rhs=xqTc[:, k, :, :],
                                start=(k == 0), stop=(k == KCp - 1),
                                perf_mode=DR)
                        nc.scalar.activation(
                            out=qk_slot(m1T, m,
                                        slice(rb * QB, (rb + 1) * QB)),
                            in_=ps, func=Act.Identity)
                    for st_ in range(QB // P):
                        rt = rb * (QB // P) + st_
                        ps = psV.tile([P, VW], F32, tag="psV")
                        for lo, hi in D_SL:
                            for k in range(KCp):
                                nc.tensor.matmul(
                                    ps[:, lo:hi],
                                    lhsT=kv8[:, k, :, rt * P:(rt + 1) * P],
                                    rhs=wvo[:, k, :, lo:hi],
                                    start=(k == 0),
                                    stop=(k == KCp - 1 and
                                          "bvo" not in nontrivial),
                                    perf_mode=DR)
                            if "bvo" in nontrivial:
                                add_bias_row(ps, "bvo", lo, hi)
                        nc.scalar.copy(qk_slot(v, rt, slice(0, D)),
                                       ps[:, 0:D])
                        nc.vector.memset(qk_slot(v, rt, slice(D, D + 1)), 1.0)

            # ====== Phase B: attention + LN1, FFN1 pipelined 1 block behind =
            with ExitStack() as ctx:
                pb = ctx.enter_context(tc.tile_pool(name=f"pb{_pe}", bufs=1))
                psS = ctx.enter_context(tc.tile_pool(name=f"psS{_pe}", bufs=2,
                                                     space="PSUM"))
                psC = ctx.enter_context(tc.tile_pool(name=f"psC{_pe}", bufs=2,
                                                     space="PSUM"))
                psH = ctx.enter_context(tc.tile_pool(name=f"psH{_pe}", bufs=2,
                                                     space="PSUM"))

                def emit_ffn1_block(fb):
                    """FFN1 + LN2 + gelu + h-store for the 4 tiles of block fb.

                    Emitted one q-block behind attention so its outT operand
                    (a DRAM transpose round trip) is long ready, and its
                    matmuls fill PE gaps in the attention eviction stretches.
                    """
                    for t in range(fb * SB, (fb + 1) * SB):
                        hpre = pb.tile([P, H], BF16, tag="hpre", bufs=2)
                        st = pb.tile([P, F1C, 6], F32, tag="st2", bufs=2)
                        for n in range(F1C):
                            ps = psH.tile([P, F1N], F32, tag="psH")
                            DCv = 1 if "ffn1" in skip else DC
                            for j in range(DCv):
                                nc.tensor.matmul(
                                    ps, lhsT=outT[:, j, t * P:(t + 1) * P],
                                    rhs=w1[:, j, n * F1N:(n + 1) * F1N],
                                    start=(j == 0),
                                    stop=(j == DCv - 1 and
                                          "bf1" not in nontrivial))
                            if "bf1" in nontrivial:
                                add_bias_row(ps, "bf1", n * F1N, (n + 1) * F1N)
                            nc.vector.tensor_copy(
                                out=hpre[:, n * F1N:(n + 1) * F1N], in_=ps)
                            nc.vector.bn_stats(st[:, n, :],
                                               hpre[:, n * F1N:(n + 1) * F1N])
                        mv = pb.tile([P, 2], F32, tag="mv2", bufs=2)
                        nc.vector.bn_aggr(mv, st)
                        rstd = emit_rsqrt(pb, nc, mv[:, 1:2], "ln2")
                        nmr = pb.tile([P, 1], F32, tag="nmr2", bufs=2)
                        nc.vector.tensor_scalar(out=nmr, in0=mv[:, 0:1],
                                                scalar1=rstd, scalar2=-1.0,
                                                op0=Alu.mult, op1=Alu.mult)
                        h_t = pb.tile([P, H], BF16, tag="h", bufs=2)
                        if "gfbf" in nontrivial:
                            tmp = pb.tile([P, H], F32, tag="lnh", bufs=2)
                            nc.vector.tensor_scalar(out=tmp, in0=hpre,
                                                    scalar1=rstd, scalar2=nmr,
                                                    op0=Alu.mult, op1=Alu.add)
                            post_ln_affine(nc, tmp, "gfbf")
                            nc.scalar.activation(out=h_t, in_=tmp,
                                                 func=Act.Gelu)
                        else:
                            # fused LN + gelu: gelu(x*rstd + (-mu*rstd))
                            nc.scalar.activation(out=h_t, in_=hpre,
                                                 func=Act.Gelu,
                                                 bias=nmr, scale=rstd)
                        nc.sync.dma_start(out=d_h.ap()[t * P:(t + 1) * P, :],
                                          in_=h_t)

                for qb in range(NB):
                    eT = pb.tile([P, RT // 2, 2, QB], F8, tag="eT", bufs=2)
                    if True:
                        for kt in range(RT):
                            ps = psS.tile([P, QB], F32, tag="psS")
                            KCv = 1 if "scores" in skip else KC
                            for c in range(KCv):
                                nc.tensor.matmul(
                                    ps, lhsT=kv8[:, c, :, kt * P:(kt + 1) * P],
                                    rhs=m1T[:, c, :, qb * QB:(qb + 1) * QB],
                                    start=(c == 0), stop=(c == KCv - 1),
                                    perf_mode=DR)
                            if has_ck:
                                nc.scalar.activation(
                                    out=qk_slot(eT, kt, slice(0, QB)), in_=ps,
                                    func=Act.Exp, scale=scale,
                                    bias=ck_t[:, kt:kt + 1])
                            else:
                                nc.scalar.activation(
                                    out=qk_slot(eT, kt, slice(0, QB)), in_=ps,
                                    func=Act.Exp, scale=scale)
                    for s in range(SB):
                        qs = qb * SB + s          # global q subtile
                        ps = psC.tile([P, D + 1], F32, tag="psC")
                        RTv = 1 if "ctx" in skip else RT // 2
                        for lo, hi in D1_SL:
                            for t_ in range(RTv):
                                nc.tensor.matmul(
                                    ps[:, lo:hi],
                                    lhsT=eT[:, t_, :, s * P:(s + 1) * P],
                                    rhs=v[:, t_, :, lo:hi],
                                    start=(t_ == 0),
                                    stop=(t_ == RTv - 1),
                                    perf_mode=DR)
                        recip = pb.tile([P, 1], F32, tag="recip", bufs=2)
                        nc.vector.reciprocal(recip, ps[:, D:D + 1])
                        xq_t = pb.tile([P, D], F32, tag="xq", bufs=3)
                        nc.sync.dma_start(out=xq_t,
                                          in_=d_xq.ap()[qs * P:(qs + 1) * P, :])
                        # r = ctxWo/esum + Xq*SW  (in-place add into r_t)
                        r_t = pb.tile([P, D], F32, tag="r", bufs=2)
                        nc.vector.tensor_scalar_mul(r_t, ps[:, 0:D], recip)
                        nc.vector.tensor_add(r_t, r_t, xq_t)
                        st = pb.tile([P, bn_dn, 6], F32, tag="st1", bufs=2)
                        for g in range(bn_dn):
                            nc.vector.bn_stats(st[:, g, :],
                                               r_t[:, g * bn_d:(g + 1) * bn_d])
                        mv = pb.tile([P, 2], F32, tag="mv1", bufs=2)
                        nc.vector.bn_aggr(mv, st)
                        rstd = emit_rsqrt(pb, nc, mv[:, 1:2], "ln1")
                        nmr = pb.tile([P, 1], F32, tag="nmr1", bufs=2)
                        nc.vector.tensor_scalar(out=nmr, in0=mv[:, 0:1],
                                                scalar1=rstd, scalar2=-1.0,
                                                op0=Alu.mult, op1=Alu.mult)
                        out_t = pb.tile([P, D], F32, tag="out", bufs=2)
                        nc.vector.tensor_scalar(out=out_t, in0=r_t, scalar1=rstd,
                                                scalar2=nmr, op0=Alu.mult,
                                                op1=Alu.add)
                        post_ln_affine(nc, out_t, "g1b1")
                        nc.sync.dma_start(out=d_outf.ap()[qs * P:(qs + 1) * P, :],
                                          in_=out_t)
                        # cast f32 -> bf16 in flight (SWDGE)
                        nc.gpsimd.dma_start(
                            out=d_outb.ap()[qs * P:(qs + 1) * P, :], in_=out_t)
                    # transpose this q-block of `out` back into SBUF for FFN1
                    for j in range(DC):
                        nc.sync.dma_start(
                            out=outT[:, j, qb * QB:(qb + 1) * QB],
                            in_=d_outb.ap()[qb * QB:(qb + 1) * QB,
                                            j * P:(j + 1) * P],
                            transpose=True)
                    if qb >= 1:
                        emit_ffn1_block(qb - 1)
                emit_ffn1_block(NB - 1)

            ab_ctx.close()  # free m1T/v + attention SBUF before FFN

            # =============== Phase C: FFN2 + LN3 ============================
            with ExitStack() as ctx:
                pc = ctx.enter_context(tc.tile_pool(name=f"pc{_pe}", bufs=1))
                psF = ctx.enter_context(tc.tile_pool(name=f"psF{_pe}", bufs=2,
                                                     space="PSUM"))
                for qb in range(NB):
                    hT = pc.tile([P, HC, QB], BF16, tag="hT", bufs=2)
                    for hc in range(HC):
                        nc.sync.dma_start(
                            out=hT[:, hc, :],
                            in_=d_h.ap()[qb * QB:(qb + 1) * QB,
                                         hc * P:(hc + 1) * P],
                            transpose=True)
                    for s in range(SB):
                        qs = qb * SB + s
                        ps = psF.tile([P, D], F32, tag="psF")
                        HCv = 1 if "ffn2" in skip else HC
                        for lo, hi in D_SL:
                            for hc in range(HCv):
                                nc.tensor.matmul(
                                    ps[:, lo:hi],
                                    lhsT=hT[:, hc, s * P:(s + 1) * P],
                                    rhs=w2[:, hc, lo:hi], start=(hc == 0),
                                    stop=(hc == HCv - 1 and
                                          "bf2" not in nontrivial))
                            if "bf2" in nontrivial:
                                add_bias_row(ps, "bf2", lo, hi)
                        o_t = pc.tile([P, D], F32, tag="oldout", bufs=3)
                        nc.sync.dma_start(out=o_t,
                                          in_=d_outf.ap()[qs * P:(qs + 1) * P, :])
                        r2 = pc.tile([P, D], F32, tag="r2", bufs=2)
                        nc.vector.tensor_add(r2, ps, o_t)
                        st3 = pc.tile([P, bn_dn, 6], F32, tag="st3", bufs=2)
                        for g in range(bn_dn):
                            nc.vector.bn_stats(st3[:, g, :],
                                               r2[:, g * bn_d:(g + 1) * bn_d])
                        mv3 = pc.tile([P, 2], F32, tag="mv3", bufs=2)
                        nc.vector.bn_aggr(mv3, st3)
                        rstd3 = emit_rsqrt(pc, nc, mv3[:, 1:2], "ln3")
                        nmr3 = pc.tile([P, 1], F32, tag="nmr3", bufs=2)
                        nc.vector.tensor_scalar(out=nmr3, in0=mv3[:, 0:1],
                                                scalar1=rstd3, scalar2=-1.0,
                                                op0=Alu.mult, op1=Alu.mult)
                        y_t = pc.tile([P, D], F32, tag="y", bufs=3)
                        nc.vector.tensor_scalar(out=y_t, in0=r2, scalar1=rstd3,
                                                scalar2=nmr3, op0=Alu.mult,
                                                op1=Alu.add)
                        post_ln_affine(nc, y_t, "g2b2")
                        nc.sync.dma_start(out=d_y.ap()[qs * P:(qs + 1) * P, :],
                                          in_=y_t)

            ectx.close()
    nc.compile()
    return nc


# ---------------------------------------------------------------------------
# SPMD runner (jit once, device-resident buffers)
# ---------------------------------------------------------------------------

class SpmdRunner:
    def __init__(self, nc, n_cores=8):
        import jax
        from jax.sharding import Mesh, PartitionSpec, NamedSharding
        from jax.experimental.shard_map import shard_map
        bass2jax.install_neuronx_cc_hook()
        self.jax = jax
        self.nc = nc
        self.n_cores = n_cores
        in_names, out_names, out_avals, zero_outs = [], [], [], []
        part = nc.partition_id_tensor.name if nc.partition_id_tensor else None
        for alloc in nc.m.functions[0].allocations:
            if not isinstance(alloc, mybir.MemoryLocationSet):
                continue
            name = alloc.memorylocations[0].name
            if alloc.kind == "ExternalInput":
                if name != part:
                    in_names.append(name)
            elif alloc.kind == "ExternalOutput":
                out_names.append(name)
                shape = tuple(alloc.tensor_shape)
                dtype = mybir.dt.np(alloc.dtype)
                out_avals.append(jax.core.ShapedArray(shape, dtype))
                zero_outs.append(np.zeros(shape, dtype))
        self.in_names = in_names
        self.out_names = out_names
        self.out_avals = out_avals
        self.zero_outs = zero_outs
        n_params = len(in_names)
        all_names = in_names + out_names + ([part] if part else [])

        def _body(*args):
            operands = list(args)
            if part is not None:
                operands.append(bass2jax.partition_id_tensor())
            return tuple(bass2jax._bass_exec_p.bind(
                *operands, out_avals=tuple(out_avals),
                in_names=tuple(all_names), out_names=tuple(out_names),
                lowering_input_output_aliases=(),
                sim_require_finite=True, sim_require_nnan=True, nc=nc))

        devices = jax.devices()[:n_cores]
        self.mesh = Mesh(np.asarray(devices), ("core",))
        in_specs = (PartitionSpec("core"),) * (n_params + len(out_names))
        out_specs = (PartitionSpec("core"),) * len(out_names)
        self.fn = jax.jit(
            shard_map(_body, mesh=self.mesh, in_specs=in_specs,
                      out_specs=out_specs, check_rep=False),
            keep_unused=True)
        self.sharding = NamedSharding(self.mesh, PartitionSpec("core"))

    def put_inputs(self, in_maps):
        concat = [np.concatenate([np.asarray(in_maps[c][n])
                                  for c in range(self.n_cores)], axis=0)
                  for n in self.in_names]
        zeros = [np.zeros((self.n_cores * z.shape[0], *z.shape[1:]), z.dtype)
                 for z in self.zero_outs]
        bufs = [self.jax.device_put(a, self.sharding) for a in concat + zeros]
        self.jax.block_until_ready(bufs)
        return bufs

    def run(self, bufs):
        outs = self.fn(*bufs)
        self.jax.block_until_ready(outs)
        return outs

    def results(self, outs):
        res = []
        for c in range(self.n_cores):
            d = {}
            for i, name in enumerate(self.out_names):
                d[name] = np.asarray(outs[i]).reshape(
                    self.n_cores, *self.out_avals[i].shape)[c]
            res.append(d)
        return res


# ---------------------------------------------------------------------------
# host entry point
# ---------------------------------------------------------------------------

_CACHE = {}


def _get_runner(nontrivial, n_elems=2):
    key = (frozenset(nontrivial), n_elems)
    if key not in _CACHE:
        nc = build_program(nontrivial=key[0], n_elems=n_elems)
        _CACHE[key] = SpmdRunner(nc, 8)
    return _CACHE[key]


def _bf16(a):
    return np.asarray(a, dtype=ml_dtypes.bfloat16)


def _f8(a):
    return np.asarray(a, dtype=ml_dtypes.float8_e4m3)


def make_weights(Wq, bq, Wk, bk, Wv, bv, Wo, bo,
                 g1, b1, W1, bf1, gf, bf, W2, bf2, g2, b2, nontrivial):
    Wq, Wk, Wv, Wo = (np.asarray(w, np.float32) for w in (Wq, Wk, Wv, Wo))
    weights = {
        "wqk": _f8((Wq @ Wk.T) * SW),
        "wvo": _f8((Wv @ Wo) * SW),
        "w1": _bf16(W1), "w2": _bf16(W2),
    }
    if "bq" in nontrivial:
        # per-key score bias: c_k = Xk@(Wk@bq) + bq.bk -- Xk-dependent part
        # is folded as an extra projection column; constant part cancels in
        # softmax only if uniform over k, which it is, so it's dropped.
        weights["ck_vec"] = np.asarray(Wk @ np.asarray(bq, np.float32),
                                       np.float32)
    if "bvo" in nontrivial:
        weights["bvo"] = _bf16(np.asarray(bv, np.float32) @ Wo * SW +
                               np.asarray(bo, np.float32) * SW)
    for nm, val in (("bf1", bf1), ("bf2", bf2)):
        if nm in nontrivial:
            weights[nm] = _bf16(val)
    for nm, g_, b_ in (("g1b1", g1, b1), ("gfbf", gf, bf), ("g2b2", g2, b2)):
        if nm in nontrivial:
            weights[nm + "_g"] = np.asarray(g_, np.float32)
            weights[nm + "_b"] = np.asarray(b_, np.float32)
    return weights


def kernel(query_modal, key_modal, Wq, bq, Wk, bk, Wv, bv, Wo, bo,
           g1, b1, W1, bf1, gf, bf, W2, bf2, g2, b2):
    query_modal = np.asarray(query_modal, np.float32)
    key_modal = np.asarray(key_modal, np.float32)
    B, N, D = query_modal.shape

    nontrivial = set()
    if not np.allclose(np.asarray(bq), 0.0):
        nontrivial.add("bq")
    if not (np.allclose(np.asarray(bv), 0.0) and
            np.allclose(np.asarray(bo), 0.0)):
        nontrivial.add("bvo")
    for nm, val in (("bf1", bf1), ("bf2", bf2)):
        if not np.allclose(np.asarray(val), 0.0):
            nontrivial.add(nm)
    for nm, g_, b_ in (("g1b1", g1, b1), ("gfbf", gf, bf), ("g2b2", g2, b2)):
        if not (np.allclose(np.asarray(g_), 1.0) and
                np.allclose(np.asarray(b_), 0.0)):
            nontrivial.add(nm)

    n_cores = 8
    n_elems = (B + n_cores - 1) // n_cores
    runner = _get_runner(frozenset(nontrivial), n_elems)

    weights = make_weights(Wq, bq, Wk, bk, Wv, bv, Wo, bo, g1, b1, W1, bf1,
                           gf, bf, W2, bf2, g2, b2, nontrivial)
    ck_vec = weights.pop("ck_vec", None)

    y = np.empty((B, N, D), np.float32)
    in_maps = []
    for c in range(n_cores):
        m = dict(weights)
        for e in range(n_elems):
            b = e * n_cores + c
            m[f"xqT{e}"] = _f8(np.ascontiguousarray(query_modal[b].T))
            m[f"xkT{e}"] = _f8(np.ascontiguousarray(key_modal[b].T))
            m[f"xq{e}"] = query_modal[b] * SW
            if ck_vec is not None:
                m[f"ck{e}"] = np.asarray(
                    (key_modal[b] @ ck_vec) / np.sqrt(D), np.float32)
        in_maps.append(m)
    bufs = runner.put_inputs(in_maps)
    outs = runner.run(bufs)
    res = runner.results(outs)
    for c in range(n_cores):
        for e in range(n_elems):
            y[e * n_cores + c] = res[c][f"y{e}"]
    return y


# revision 31
# speedup vs baseline: 4.3739x; 4.3739x over previous
"""Trainium2 Bass kernel for CrossModalAttentionImproved.

Single-head cross attention + FFN transformer block:
  q = Xq@Wq+bq; k = Xk@Wk+bk; v = Xk@Wv+bv
  attn = softmax(q k^T / sqrt(D)); ctx = attn@v
  out = LN(Xq + ctx@Wo + bo; g1,b1)
  h = gelu(LN(out@W1 + bf1; gf,bf))
  y = LN(out + h@W2 + bf2; g2,b2)

Sharding: data-parallel over batch. B=16 across 8 cores x 2 sequential
elements per core (one batch element per program iteration). Params
replicated.

Algebraic folds (host precomputes, exact):
  - scores = q k^T = Xq (Wq Wk^T) Xk^T: one fused projection M1 = Xq@Wqk,
    and Xk^T (the raw fp8 input) is the scores operand directly -> the
    K projection disappears. bk cancels in softmax; bq becomes a per-key
    bias column folded into the exp (zero here).
  - ctx@Wo = attn @ (V Wo) with VWo = Xk@(Wv Wo): the Wo stage, the ctx
    PE-transposes and fp8 requant all disappear.
  - fp8 weights are scaled x64 host-side (escapes e4m3 subnormals);
    the scale is divided out via the folded exp constant / pre-scaled
    residual (Xq*64 from host), so no extra device ops.

Layout strategy (attn matmuls fp8 DoubleRow, FFN bf16, fp32 PSUM):
  - host pre-transposes Xq/Xk to [D,N] fp8
  - M1T produced transposed [D,N] via PE; V=Xk@WvWo natural [N,D] with a
    ones column -> softmax sums land as a per-partition column
  - scoresT[k,q] per k-tile; exp on ACT (scale folded); eT fp8
  - ctx eviction: reciprocal + fused (ps*recip + Xq') on DVE, LN1 via
    bn_stats + DVE-bit-trick rsqrt
  - FFN1 natural out; LN+GELU fused into a single ACT op
  - h bounced through DRAM with DMA-transpose (2-byte xbar) for FFN2
"""

import sys

if '/opt/trn_rl_repo' not in sys.path:
    sys.path.insert(0, '/opt/trn_rl_repo')

import math
from contextlib import ExitStack

import numpy as np
import ml_dtypes

import concourse.bass as bass
import concourse.tile as tile
from concourse import bacc, mybir
from concourse import bass2jax

F32 = mybir.dt.float32
BF16 = mybir.dt.bfloat16
F8 = mybir.dt.float8e4
DR = mybir.MatmulPerfMode.DoubleRow
U32 = mybir.dt.uint32
Alu = mybir.AluOpType
Act = mybir.ActivationFunctionType

EPS = 1e-5
P = 128
SW = 64.0          # host-side scale on fused fp8 weights (and Xq residual)


# ---------------------------------------------------------------------------
# device program
# ---------------------------------------------------------------------------

def build_program(N=2048, D=768, H=3072, QB=512, n_elems=1, n_reps=1,
                  nontrivial=frozenset(), skip=frozenset()):
    """Build + compile the per-core program (n_elems batch elements).

    nontrivial: subset of {bq, bvo, bf1, bf2, g1b1, gfbf, g2b2} naming
    affine params that are not identity and need real ops emitted.
    skip: phase-ablation set for benchmarking, subset of
    {proj, scores, ctx, ffn1, ffn2}.
    """
    DC = D // P          # d chunks (6)
    HC = H // P          # h chunks (24)
    RT = N // P          # row tiles (16)
    NB = N // QB         # q blocks (4)
    SB = QB // P         # subtiles per block (4)
    F1N = min(512, H)    # FFN1 n-chunk width
    F1C = H // F1N       # FFN1 n-chunks (6)
    KC = DC // 2         # fp8 DR d chunk pairs (3)
    scale = 1.0 / (math.sqrt(D) * SW)
    has_ck = "bq" in nontrivial      # per-key score bias column
    VW = D + 16          # v tile row width (D + ones col + optional ck)

    def slices(total):
        out, lo = [], 0
        while lo < total:
            hi = min(lo + 512, total)
            out.append((lo, hi))
            lo = hi
        return out

    D_SL = slices(D)          # [(0,512),(512,768)]
    D1_SL = slices(D + 1)     # [(0,512),(512,769)]

    nc = bacc.Bacc("TRN2", target_bir_lowering=False, debug=False,
                   num_devices=8)

    # ---- DRAM I/O -----------------------------------------------------
    l_xqT = [nc.dram_tensor(f"xqT{e}", [D, N], F8, kind="ExternalInput")
             for e in range(n_elems)]
    l_xkT = [nc.dram_tensor(f"xkT{e}", [D, N], F8, kind="ExternalInput")
             for e in range(n_elems)]
    l_xq = [nc.dram_tensor(f"xq{e}", [N, D], F32, kind="ExternalInput")
            for e in range(n_elems)]
    d_wqk = nc.dram_tensor("wqk", [D, D], F8, kind="ExternalInput")
    d_wvo = nc.dram_tensor("wvo", [D, D], F8, kind="ExternalInput")
    d_w1 = nc.dram_tensor("w1", [D, H], BF16, kind="ExternalInput")
    d_w2 = nc.dram_tensor("w2", [H, D], BF16, kind="ExternalInput")
    dram_aff = {}
    l_ck = []
    if has_ck:
        l_ck = [nc.dram_tensor(f"ck{e}", [N], F32, kind="ExternalInput")
                for e in range(n_elems)]
    for nm, sz in (("bvo", D), ("bf1", H), ("bf2", D)):
        if nm in nontrivial:
            dram_aff[nm] = nc.dram_tensor(nm, [sz], BF16, kind="ExternalInput")
    for nm, sz in (("g1b1", D), ("gfbf", H), ("g2b2", D)):
        if nm in nontrivial:
            dram_aff[nm + "_g"] = nc.dram_tensor(nm + "_g", [sz], F32,
                                                 kind="ExternalInput")
            dram_aff[nm + "_b"] = nc.dram_tensor(nm + "_b", [sz], F32,
                                                 kind="ExternalInput")
    l_y = [nc.dram_tensor(f"y{e}", [N, D], F32, kind="ExternalOutput")
           for e in range(n_elems)]
    # internal scratch
    l_outf = [nc.dram_tensor(f"out_f32_{e}", [N, D], F32)
              for e in range(n_elems)]
    l_outb = [nc.dram_tensor(f"out_b16_{e}", [N, D], BF16)
              for e in range(n_elems)]
    l_h = [nc.dram_tensor(f"h_b16_{e}", [N, H], BF16)
           for e in range(n_elems)]

    # bn_stats subgroup sizes
    bn_d = math.gcd(512, D)      # 256 for 768
    bn_dn = D // bn_d

    def emit_rsqrt(pool, nc, var_ap, tag):
        """rstd[P,1] f32 = 1/sqrt(var+EPS), DVE only (no ACT tables)."""
        ve = pool.tile([P, 1], F32, tag=f"rs_ve_{tag}", bufs=2)
        nc.vector.tensor_scalar_add(ve, var_ap, EPS)
        y = pool.tile([P, 1], F32, tag=f"rs_y_{tag}", bufs=2)
        nc.vector.tensor_scalar(
            out=y.bitcast(U32), in0=ve.bitcast(U32),
            scalar1=1, scalar2=0xFFFFFFFF,
            op0=Alu.logical_shift_right, op1=Alu.bitwise_xor)
        nc.vector.tensor_scalar(
            out=y.bitcast(U32), in0=y.bitcast(U32),
            scalar1=0xA0C8A620, scalar2=None, op0=Alu.subtract)
        t = pool.tile([P, 1], F32, tag=f"rs_t_{tag}", bufs=2)
        for _ in range(1):
            nc.vector.tensor_mul(t, y, y)
            nc.vector.tensor_mul(t, t, ve)
            nc.vector.tensor_scalar(out=t, in0=t, scalar1=-0.5, scalar2=1.5,
                                    op0=Alu.mult, op1=Alu.add)
            nc.vector.tensor_mul(y, y, t)
        return y

    def load_rep(pool, nc, dram, sz, tag):
        """Broadcast a [sz] dram vector across partitions -> [P, sz] tile."""
        t = pool.tile([P, sz], dram.dtype, tag=tag, bufs=1, name=tag)
        ap = dram.ap()
        bcast = bass.AP(tensor=ap.tensor, offset=ap.offset,
                        ap=[[0, P]] + list(ap.ap))
        nc.gpsimd.dma_start(out=t, in_=bcast)
        return t

    with tile.TileContext(nc) as tc, ExitStack() as octx:
        pers = octx.enter_context(tc.tile_pool(name="pers", bufs=1))

        # fused attention weights, DR pair layout: d = c*256 + i*128 + p
        wqk = pers.tile([P, KC, 2, D], F8)
        nc.sync.dma_start(out=wqk, in_=d_wqk.ap().rearrange(
            "(c i p) f -> p c i f", p=P, i=2))
        wvo = pers.tile([P, KC, 2, D], F8)
        nc.sync.dma_start(out=wvo, in_=d_wvo.ap().rearrange(
            "(c i p) f -> p c i f", p=P, i=2))
        # FFN weights, persistent across elements. Loaded on the Pool
        # (SWDGE) queue in per-chunk pieces so they never block the SP
        # queue's latency-critical phase-A streams; they have all of
        # phases A+B to land before first use in phase C.
        w1 = pers.tile([P, DC, H], BF16)
        for j in range(DC):
            nc.gpsimd.dma_start(out=w1[:, j, :], in_=d_w1.ap().rearrange(
                "(c p) f -> p c f", p=P)[:, j, :])
        w2 = pers.tile([P, HC, D], BF16)
        for j in range(0, HC, 4):
            nc.gpsimd.dma_start(
                out=w2[:, j:j + 4, :], in_=d_w2.ap().rearrange(
                    "(c p) f -> p c f", p=P)[:, j:j + 4, :])


        ones_row = None
        if any(k in nontrivial for k in ("bvo", "bf1", "bf2")):
            ones_row = pers.tile([1, P], BF16)
            nc.vector.memset(ones_row, 1.0)
        bias_rows = {}
        for nm in ("bvo", "bf1", "bf2"):
            if nm in nontrivial:
                sz = H if nm == "bf1" else D
                t = pers.tile([1, sz], BF16, tag=f"brow_{nm}",
                              name=f"brow_{nm}")
                nc.sync.dma_start(out=t, in_=dram_aff[nm].ap().rearrange(
                    "(o f) -> o f", o=1))
                bias_rows[nm] = t
        gain_reps = {}
        for nm in ("g1b1", "gfbf", "g2b2"):
            if nm in nontrivial:
                sz = H if nm == "gfbf" else D
                gain_reps[nm + "_g"] = load_rep(pers, nc, dram_aff[nm + "_g"],
                                                sz, f"grep_{nm}")
                gain_reps[nm + "_b"] = load_rep(pers, nc, dram_aff[nm + "_b"],
                                                sz, f"brep_{nm}")

        def add_bias_row(psum_ap, nm, lo, hi):
            """Accumulate broadcast bias row into psum via K=1 matmul."""
            nc.tensor.matmul(psum_ap[:, lo:hi], lhsT=ones_row,
                             rhs=bias_rows[nm][:, lo:hi],
                             start=False, stop=True)

        def post_ln_affine(nc, buf, nm):
            if nm in nontrivial:
                nc.vector.tensor_mul(buf, buf, gain_reps[nm + "_g"])
                nc.vector.tensor_add(buf, buf, gain_reps[nm + "_b"])

        iters = [(r, e) for r in range(n_reps) for e in range(n_elems)]
        for _idx, (_rep, _e) in enumerate(iters):
            _pe = f"{_rep}_{_e}"
            d_xqT, d_xkT, d_xq = l_xqT[_e], l_xkT[_e], l_xq[_e]
            d_y, d_outf, d_outb, d_h = l_y[_e], l_outf[_e], l_outb[_e], l_h[_e]
            # ==== cross-phase pool: outT (B->C) ====
            ectx = ExitStack()
            crossBC = ectx.enter_context(tc.tile_pool(name=f"crossBC{_pe}",
                                                      bufs=1))
            outT = crossBC.tile([P, DC, N], BF16)

            # ============ Phases A+B share kv8 / m1T / v tiles ============
            ab_ctx = ectx.enter_context(ExitStack())
            qkv = ab_ctx.enter_context(tc.tile_pool(name=f"qkv{_pe}", bufs=1))
            kv8 = qkv.tile([P, KC, 2, N], F8)      # Xk^T resident
            m1T = qkv.tile([P, KC, 2, N], F8)      # (Xq@Wqk)^T
            v = qkv.tile([P, RT // 2, 2, VW], F8)  # Xk@WvWo | ones
            ck_t = qkv.tile([P, RT], F32) if has_ck else None

            def qk_slot(t, m, sl):
                return t[:, m // 2, m % 2, sl]

            with ExitStack() as ctx:
                pa = ctx.enter_context(tc.tile_pool(name=f"pa{_pe}", bufs=1))
                psA = ctx.enter_context(tc.tile_pool(name=f"psA{_pe}", bufs=3,
                                                     space="PSUM"))
                psV = ctx.enter_context(tc.tile_pool(name=f"psV{_pe}", bufs=2,
                                                     space="PSUM"))

                def x_dram(d):
                    return d.ap().rearrange("(c i p) n -> p c i n", p=P, i=2)

                # kv8 loaded in column chunks so V(rt) starts early
                for cb in range(NB):
                    nc.sync.dma_start(
                        out=kv8[:, :, :, cb * QB:(cb + 1) * QB],
                        in_=x_dram(d_xkT)[:, :, :, cb * QB:(cb + 1) * QB])
                if has_ck:
                    nc.sync.dma_start(out=ck_t, in_=l_ck[_e].ap().rearrange(
                        "(t p) -> p t", p=P))

                # Interleaved per row-block: M1T chunk (phase B needs it
                # first) then this block's V row tiles (overlap xqT stream)
                KCp = 1 if "proj" in skip else KC
                for rb in range(NB):
                    xqTc = pa.tile([P, KC, 2, QB], F8, tag="xqTc",
                                   bufs=2, name="xqTc")
                    nc.sync.dma_start(out=xqTc, in_=x_dram(d_xqT)[
                        :, :, :, rb * QB:(rb + 1) * QB])
                    for m in range(DC):
                        ps = psA.tile([P, QB], F32, tag="psA")
                        for k in range(KCp):
                            nc.tensor.matmul(
                                ps,
                                lhsT=wqk[:, k, :, m * P:(m + 1) * P],
                                rhs=xqTc[:, k, :, :],
                                start=(k == 0), stop=(k == KCp - 1),
                                perf_mode=DR)
                        nc.scalar.activation(
                            out=qk_slot(m1T, m,
                                        slice(rb * QB, (rb + 1) * QB)),
                            in_=ps, func=Act.Identity)
                    for st_ in range(QB // P):
                        rt = rb * (QB // P) + st_
                        ps = psV.tile([P, VW], F32, tag="psV")
                        for lo, hi in D_SL:
                            for k in range(KCp):
                                nc.tensor.matmul(
                                    ps[:, lo:hi],
                                    lhsT=kv8[:, k, :, rt * P:(rt + 1) * P],
                                    rhs=wvo[:, k, :, lo:hi],
                                    start=(k == 0),
                                    stop=(k == KCp - 1 and
                                          "bvo" not in nontrivial),
                                    perf_mode=DR)
                            if "bvo" in nontrivial:
                                add_bias_row(ps, "bvo", lo, hi)
                        nc.scalar.copy(qk_slot(v, rt, slice(0, D)),
                                       ps[:, 0:D])
                        nc.vector.memset(qk_slot(v, rt, slice(D, D + 1)), 1.0)

            # ====== Phase B: attention + LN1, FFN1 pipelined 1 block behind =
            with ExitStack() as ctx:
                pb = ctx.enter_context(tc.tile_pool(name=f"pb{_pe}", bufs=1))
                psS = ctx.enter_context(tc.tile_pool(name=f"psS{_pe}", bufs=2,
                                                     space="PSUM"))
                psC = ctx.enter_context(tc.tile_pool(name=f"psC{_pe}", bufs=2,
                                                     space="PSUM"))
                psH = ctx.enter_context(tc.tile_pool(name=f"psH{_pe}", bufs=2,
                                                     space="PSUM"))

                def emit_ffn1_block(fb):
                    """FFN1 + LN2 + gelu + h-store for the 4 tiles of block fb.

                    Emitted one q-block behind attention so its outT operand
                    (a DRAM transpose round trip) is long ready, and its
                    matmuls fill PE gaps in the attention eviction stretches.
                    """
                    for t in range(fb * SB, (fb + 1) * SB):
                        hpre = pb.tile([P, H], BF16, tag="hpre", bufs=2)
                        st = pb.tile([P, F1C, 6], F32, tag="st2", bufs=2)
                        for n in range(F1C):
                            ps = psH.tile([P, F1N], F32, tag="psH")
                            DCv = 1 if "ffn1" in skip else DC
                            for j in range(DCv):
                                nc.tensor.matmul(
                                    ps, lhsT=outT[:, j, t * P:(t + 1) * P],
                                    rhs=w1[:, j, n * F1N:(n + 1) * F1N],
                                    start=(j == 0),
                                    stop=(j == DCv - 1 and
                                          "bf1" not in nontrivial))
                            if "bf1" in nontrivial:
                                add_bias_row(ps, "bf1", n * F1N, (n + 1) * F1N)
                            nc.vector.tensor_copy(
                                out=hpre[:, n * F1N:(n + 1) * F1N], in_=ps)
                            nc.vector.bn_stats(st[:, n, :],
                                               hpre[:, n * F1N:(n + 1) * F1N])
                        mv = pb.tile([P, 2], F32, tag="mv2", bufs=2)
                        nc.vector.bn_aggr(mv, st)
                        rstd = emit_rsqrt(pb, nc, mv[:, 1:2], "ln2")
                        nmr = pb.tile([P, 1], F32, tag="nmr2", bufs=2)
                        nc.vector.tensor_scalar(out=nmr, in0=mv[:, 0:1],
                                                scalar1=rstd, scalar2=-1.0,
                                                op0=Alu.mult, op1=Alu.mult)
                        h_t = pb.tile([P, H], BF16, tag="h", bufs=2)
                        if "gfbf" in nontrivial:
                            tmp = pb.tile([P, H], F32, tag="lnh", bufs=2)
                            nc.vector.tensor_scalar(out=tmp, in0=hpre,
                                                    scalar1=rstd, scalar2=nmr,
                                                    op0=Alu.mult, op1=Alu.add)
                            post_ln_affine(nc, tmp, "gfbf")
                            nc.scalar.activation(out=h_t, in_=tmp,
                                                 func=Act.Gelu)
                        else:
                            # fused LN + gelu: gelu(x*rstd + (-mu*rstd))
                            nc.scalar.activation(out=h_t, in_=hpre,
                                                 func=Act.Gelu,
                                                 bias=nmr, scale=rstd)
                        nc.sync.dma_start(out=d_h.ap()[t * P:(t + 1) * P, :],
                                          in_=h_t)

                for qb in range(NB):
                    eT = pb.tile([P, RT // 2, 2, QB], F8, tag="eT", bufs=2)
                    if True:
                        for kt in range(RT):
                            ps = psS.tile([P, QB], F32, tag="psS")
                            KCv = 1 if "scores" in skip else KC
                            for c in range(KCv):
                                nc.tensor.matmul(
                                    ps, lhsT=kv8[:, c, :, kt * P:(kt + 1) * P],
                                    rhs=m1T[:, c, :, qb * QB:(qb + 1) * QB],
                                    start=(c == 0), stop=(c == KCv - 1),
                                    perf_mode=DR)
                            if has_ck:
                                nc.scalar.activation(
                                    out=qk_slot(eT, kt, slice(0, QB)), in_=ps,
                                    func=Act.Exp, scale=scale,
                                    bias=ck_t[:, kt:kt + 1])
                            else:
                                nc.scalar.activation(
                                    out=qk_slot(eT, kt, slice(0, QB)), in_=ps,
                                    func=Act.Exp, scale=scale)
                    for s in range(SB):
                        qs = qb * SB + s          # global q subtile
                        ps = psC.tile([P, D + 1], F32, tag="psC")
                        RTv = 1 if "ctx" in skip else RT // 2
                        for lo, hi in D1_SL:
                            for t_ in range(RTv):
                                nc.tensor.matmul(
                                    ps[:, lo:hi],
                                    lhsT=eT[:, t_, :, s * P:(s + 1) * P],
                                    rhs=v[:, t_, :, lo:hi],
                                    start=(t_ == 0),
                                    stop=(t_ == RTv - 1),
                                    perf_mode=DR)
                        recip = pb.tile([P, 1], F32, tag="recip", bufs=2)
                        nc.vector.reciprocal(recip, ps[:, D:D + 1])
                        xq_t = pb.tile([P, D], F32, tag="xq", bufs=3)
                        nc.scalar.dma_start(out=xq_t,
                                            in_=d_xq.ap()[qs * P:(qs + 1) * P, :])
                        # r = ctxWo/esum + Xq*SW  (in-place add into r_t)
                        r_t = pb.tile([P, D], F32, tag="r", bufs=2)
                        nc.vector.tensor_scalar_mul(r_t, ps[:, 0:D], recip)
                        nc.vector.tensor_add(r_t, r_t, xq_t)
                        st = pb.tile([P, bn_dn, 6], F32, tag="st1", bufs=2)
                        for g in range(bn_dn):
                            nc.vector.bn_stats(st[:, g, :],
                                               r_t[:, g * bn_d:(g + 1) * bn_d])
                        mv = pb.tile([P, 2], F32, tag="mv1", bufs=2)
                        nc.vector.bn_aggr(mv, st)
                        rstd = emit_rsqrt(pb, nc, mv[:, 1:2], "ln1")
                        nmr = pb.tile([P, 1], F32, tag="nmr1", bufs=2)
                        nc.vector.tensor_scalar(out=nmr, in0=mv[:, 0:1],
                                                scalar1=rstd, scalar2=-1.0,
                                                op0=Alu.mult, op1=Alu.mult)
                        out_t = pb.tile([P, D], F32, tag="out", bufs=2)
                        nc.vector.tensor_scalar(out=out_t, in0=r_t, scalar1=rstd,
                                                scalar2=nmr, op0=Alu.mult,
                                                op1=Alu.add)
                        post_ln_affine(nc, out_t, "g1b1")
                        nc.gpsimd.dma_start(
                            out=d_outf.ap()[qs * P:(qs + 1) * P, :], in_=out_t)
                        # cast f32 -> bf16 in flight (SWDGE)
                        nc.gpsimd.dma_start(
                            out=d_outb.ap()[qs * P:(qs + 1) * P, :], in_=out_t)
                    # transpose this q-block of `out` back into SBUF for FFN1
                    for j in range(DC):
                        nc.sync.dma_start(
                            out=outT[:, j, qb * QB:(qb + 1) * QB],
                            in_=d_outb.ap()[qb * QB:(qb + 1) * QB,
                                            j * P:(j + 1) * P],
                            transpose=True)
                    if qb >= 1:
                        emit_ffn1_block(qb - 1)
                emit_ffn1_block(NB - 1)

            ab_ctx.close()  # free m1T/v + attention SBUF before FFN

            # =============== Phase C: FFN2 + LN3 ============================
            with ExitStack() as ctx:
                pc = ctx.enter_context(tc.tile_pool(name=f"pc{_pe}", bufs=1))
                psF = ctx.enter_context(tc.tile_pool(name=f"psF{_pe}", bufs=2,
                                                     space="PSUM"))
                for qb in range(NB):
                    hT = pc.tile([P, HC, QB], BF16, tag="hT", bufs=2)
                    for hc in range(HC):
                        nc.sync.dma_start(
                            out=hT[:, hc, :],
                            in_=d_h.ap()[qb * QB:(qb + 1) * QB,
                                         hc * P:(hc + 1) * P],
                            transpose=True)
                    for s in range(SB):
                        qs = qb * SB + s
                        ps = psF.tile([P, D], F32, tag="psF")
                        HCv = 1 if "ffn2" in skip else HC
                        for lo, hi in D_SL:
                            for hc in range(HCv):
                                nc.tensor.matmul(
                                    ps[:, lo:hi],
                                    lhsT=hT[:, hc, s * P:(s + 1) * P],
                                    rhs=w2[:, hc, lo:hi], start=(hc == 0),
                                    stop=(hc == HCv - 1 and
                                          "bf2" not in nontrivial))
                            if "bf2" in nontrivial:
                                add_bias_row(ps, "bf2", lo, hi)
                        o_t = pc.tile([P, D], F32, tag="oldout", bufs=3)
                        nc.scalar.dma_start(out=o_t,
                                            in_=d_outf.ap()[qs * P:(qs + 1) * P, :])
                        r2 = pc.tile([P, D], F32, tag="r2", bufs=2)
                        nc.vector.tensor_add(r2, ps, o_t)
                        st3 = pc.tile([P, bn_dn, 6], F32, tag="st3", bufs=2)
                        for g in range(bn_dn):
                            nc.vector.bn_stats(st3[:, g, :],
                                               r2[:, g * bn_d:(g + 1) * bn_d])
                        mv3 = pc.tile([P, 2], F32, tag="mv3", bufs=2)
                        nc.vector.bn_aggr(mv3, st3)
                        rstd3 = emit_rsqrt(pc, nc, mv3[:, 1:2], "ln3")
                        nmr3 = pc.tile([P, 1], F32, tag="nmr3", bufs=2)
                        nc.vector.tensor_scalar(out=nmr3, in0=mv3[:, 0:1],
                                                scalar1=rstd3, scalar2=-1.0,
                                                op0=Alu.mult, op1=Alu.mult)
                        y_t = pc.tile([P, D], F32, tag="y", bufs=3)
                        nc.vector.tensor_scalar(out=y_t, in0=r2, scalar1=rstd3,
                                                scalar2=nmr3, op0=Alu.mult,
                                                op1=Alu.add)
                        post_ln_affine(nc, y_t, "g2b2")
                        nc.sync.dma_start(out=d_y.ap()[qs * P:(qs + 1) * P, :],
                                          in_=y_t)

            ectx.close()
    nc.compile()
    return nc


# ---------------------------------------------------------------------------
# SPMD runner (jit once, device-resident buffers)
# ---------------------------------------------------------------------------

class SpmdRunner:
    def __init__(self, nc, n_cores=8):
        import jax
        from jax.sharding import Mesh, PartitionSpec, NamedSharding
        from jax.experimental.shard_map import shard_map
        bass2jax.install_neuronx_cc_hook()
        self.jax = jax
        self.nc = nc
        self.n_cores = n_cores
        in_names, out_names, out_avals, zero_outs = [], [], [], []
        part = nc.partition_id_tensor.name if nc.partition_id_tensor else None
        for alloc in nc.m.functions[0].allocations:
            if not isinstance(alloc, mybir.MemoryLocationSet):
                continue
            name = alloc.memorylocations[0].name
            if alloc.kind == "ExternalInput":
                if name != part:
                    in_names.append(name)
            elif alloc.kind == "ExternalOutput":
                out_names.append(name)
                shape = tuple(alloc.tensor_shape)
                dtype = mybir.dt.np(alloc.dtype)
                out_avals.append(jax.core.ShapedArray(shape, dtype))
                zero_outs.append(np.zeros(shape, dtype))
        self.in_names = in_names
        self.out_names = out_names
        self.out_avals = out_avals
        self.zero_outs = zero_outs
        n_params = len(in_names)
        all_names = in_names + out_names + ([part] if part else [])

        def _body(*args):
            operands = list(args)
            if part is not None:
                operands.append(bass2jax.partition_id_tensor())
            return tuple(bass2jax._bass_exec_p.bind(
                *operands, out_avals=tuple(out_avals),
                in_names=tuple(all_names), out_names=tuple(out_names),
                lowering_input_output_aliases=(),
                sim_require_finite=True, sim_require_nnan=True, nc=nc))

        devices = jax.devices()[:n_cores]
        self.mesh = Mesh(np.asarray(devices), ("core",))
        in_specs = (PartitionSpec("core"),) * (n_params + len(out_names))
        out_specs = (PartitionSpec("core"),) * len(out_names)
        self.fn = jax.jit(
            shard_map(_body, mesh=self.mesh, in_specs=in_specs,
                      out_specs=out_specs, check_rep=False),
            keep_unused=True)
        self.sharding = NamedSharding(self.mesh, PartitionSpec("core"))

    def put_inputs(self, in_maps):
        concat = [np.concatenate([np.asarray(in_maps[c][n])
                                  for c in range(self.n_cores)], axis=0)
                  for n in self.in_names]
        zeros = [np.zeros((self.n_cores * z.shape[0], *z.shape[1:]), z.dtype)
                 for z in self.zero_outs]
        bufs = [self.jax.device_put(a, self.sharding) for a in concat + zeros]
        self.jax.block_until_ready(bufs)
        return bufs

    def run(self, bufs):
        outs = self.fn(*bufs)
        self.jax.block_until_ready(outs)
        return outs

    def results(self, outs):
        res = []
        for c in range(self.n_cores):
            d = {}
            for i, name in enumerate(self.out_names):
                d[name] = np.asarray(outs[i]).reshape(
                    self.n_cores, *self.out_avals[i].shape)[c]
            res.append(d)
        return res


# ---------------------------------------------------------------------------
# host entry point
# ---------------------------------------------------------------------------

_CACHE = {}


def _get_runner(nontrivial, n_elems=2):
    key = (frozenset(nontrivial), n_elems)
    if key not in _CACHE:
        nc = build_program(nontrivial=key[0], n_elems=n_elems)
        _CACHE[key] = SpmdRunner(nc, 8)
    return _CACHE[key]


def _bf16(a):
    return np.asarray(a, dtype=ml_dtypes.bfloat16)


def _f8(a):
    return np.asarray(a, dtype=ml_dtypes.float8_e4m3)


def make_weights(Wq, bq, Wk, bk, Wv, bv, Wo, bo,
                 g1, b1, W1, bf1, gf, bf, W2, bf2, g2, b2, nontrivial):
    Wq, Wk, Wv, Wo = (np.asarray(w, np.float32) for w in (Wq, Wk, Wv, Wo))
    weights = {
        "wqk": _f8((Wq @ Wk.T) * SW),
        "wvo": _f8((Wv @ Wo) * SW),
        "w1": _bf16(W1), "w2": _bf16(W2),
    }
    if "bq" in nontrivial:
        # per-key score bias: c_k = Xk@(Wk@bq) + bq.bk -- Xk-dependent part
        # is folded as an extra projection column; constant part cancels in
        # softmax only if uniform over k, which it is, so it's dropped.
        weights["ck_vec"] = np.asarray(Wk @ np.asarray(bq, np.float32),
                                       np.float32)
    if "bvo" in nontrivial:
        weights["bvo"] = _bf16(np.asarray(bv, np.float32) @ Wo * SW +
                               np.asarray(bo, np.float32) * SW)
    for nm, val in (("bf1", bf1), ("bf2", bf2)):
        if nm in nontrivial:
            weights[nm] = _bf16(val)
    for nm, g_, b_ in (("g1b1", g1, b1), ("gfbf", gf, bf), ("g2b2", g2, b2)):
        if nm in nontrivial:
            weights[nm + "_g"] = np.asarray(g_, np.float32)
            weights[nm + "_b"] = np.asarray(b_, np.float32)
    return weights


def kernel(query_modal, key_modal, Wq, bq, Wk, bk, Wv, bv, Wo, bo,
           g1, b1, W1, bf1, gf, bf, W2, bf2, g2, b2):
    query_modal = np.asarray(query_modal, np.float32)
    key_modal = np.asarray(key_modal, np.float32)
    B, N, D = query_modal.shape

    nontrivial = set()
    if not np.allclose(np.asarray(bq), 0.0):
        nontrivial.add("bq")
    if not (np.allclose(np.asarray(bv), 0.0) and
            np.allclose(np.asarray(bo), 0.0)):
        nontrivial.add("bvo")
    for nm, val in (("bf1", bf1), ("bf2", bf2)):
        if not np.allclose(np.asarray(val), 0.0):
            nontrivial.add(nm)
    for nm, g_, b_ in (("g1b1", g1, b1), ("gfbf", gf, bf), ("g2b2", g2, b2)):
        if not (np.allclose(np.asarray(g_), 1.0) and
                np.allclose(np.asarray(b_), 0.0)):
            nontrivial.add(nm)

    n_cores = 8
    n_elems = (B + n_cores - 1) // n_cores
    runner = _get_runner(frozenset(nontrivial), n_elems)

    weights = make_weights(Wq, bq, Wk, bk, Wv, bv, Wo, bo, g1, b1, W1, bf1,
                           gf, bf, W2, bf2, g2, b2, nontrivial)
    ck_vec = weights.pop("ck_vec", None)

    y = np.empty((B, N, D), np.float32)
    in_maps = []
    for c in range(n_cores):
        m = dict(weights)
        for e in range(n_elems):
            b = e * n_cores + c
            m[f"xqT{e}"] = _f8(np.ascontiguousarray(query_modal[b].T))
            m[f"xkT{e}"] = _f8(np.ascontiguousarray(key_modal[b].T))
            m[f"xq{e}"] = query_modal[b] * SW
            if ck_vec is not None:
                m[f"ck{e}"] = np.asarray(
                    (key_modal[b] @ ck_vec) / np.sqrt(D), np.float32)
        in_maps.append(m)
    bufs = runner.put_inputs(in_maps)
    outs = runner.run(bufs)
    res = runner.results(outs)
    for c in range(n_cores):
        for e in range(n_elems):
            y[e * n_cores + c] = res[c][f"y{e}"]
    return y
